# revision 1
# baseline (speedup 1.0000x reference)
"""CustomLSTM cell (4 gated projections + cell update) on 8 TRN2 NeuronCores.

Data-parallel over the batch dim: each core processes B/8 = 4096 rows.
Per core, z = x @ [Wi|Wf|Wg|Wo] is computed as bf16 matmuls accumulating
f32 into 4 PSUM banks (N = 4*512), the gate nonlinearities run on the
scalar engine straight out of PSUM, and the cell/hidden updates run on
the vector engine.  Host-side prep lays x out transposed ([p, ko, b]
per 512-row batch group) so every DMA is contiguous per partition, and
casts x/W to bf16 (PSUM accumulation stays f32).

Self-contained: shapes/sharding hardcoded for
input [32768, 1024], cell_state [32768, 512], W* [1024, 512].
"""

import os

import numpy as np
import ml_dtypes

import bass_rust
import concourse.bass as bass
import concourse.mybir as mybir
import concourse.tile as tile
from concourse.bass_utils import run_bass_kernel_spmd

N_CORES = 8
B = 32768
D = 1024
H = 512
P = 128
B_LOC = B // N_CORES        # 4096 rows per core
KO = D // P                 # 8 k-subtiles
NW = 4 * H                  # 2048 concatenated gate dim
NG = NW // H                # 4 psum banks of 512
BG_ROWS = 512               # batch rows per x slab
BG = B_LOC // BG_ROWS       # 8 slabs per core
BT_PER_BG = BG_ROWS // P    # 4 batch tiles per slab

BF16 = mybir.dt.bfloat16
F32 = mybir.dt.float32

# Filled by the last kernel() call: BassKernelResults (exec_time_ns etc).
LAST_RESULTS = None
_CACHED = {}


def _split_multi_waits(nc):
    """Legalize for a walrus build that accepts one sync-wait per instruction.

    Tile's wait assignment attaches every needed sem wait to the consuming
    instruction; this backend rejects >1 ("Too many sync wait commands").
    Move all but the last wait onto dedicated NoOps inserted just before the
    instruction on the same engine queue — sequential waits on one engine are
    equivalent to a single multi-wait instruction for monotone sem waits.
    """
    n = 0
    for f in nc.m.functions:
        for blk in f.blocks:
            insts = blk.instructions
            if not any(
                i.sync_info is not None and len(i.sync_info.on_wait) > 1
                for i in insts
            ):
                continue
            out = []
            for inst in insts:
                si = inst.sync_info
                if si is not None and len(si.on_wait) > 1:
                    waits = list(si.on_wait)
                    for w in waits[:-1]:
                        nop = mybir.InstNoOp(name=f"waitsplit_{n}", ins=[], outs=[])
                        n += 1
                        nop.engine = inst.engine
                        nop.sync_info = bass_rust.SyncInfo(on_wait=[w], on_update=[])
                        out.append(nop)
                    inst.sync_info = bass_rust.SyncInfo(
                        on_wait=[waits[-1]], on_update=list(si.on_update)
                    )
                out.append(inst)
            blk.instructions = out


class _FastTailTileContext(tile.TileContext):
    """Drop the second tail all-engine barrier.

    The stock tail is [drain+waits][barrier][sem/queue reset][barrier]; the
    final barrier only isolates the gpsimd-side reset from code that would
    follow it — nothing follows it here, and NRT waits for every engine
    stream (including gpsimd's reset) to halt before completion, so engines
    can end right after the first barrier. Saves ~4-6us of EVSEM ring.
    """

    def _drain_and_barrier(self, tick_clock, wait_clock):
        from concourse.vector_clock import ScopedClock

        drain_inst = self.nc.sync.drain()
        wait_clock.add_sem_waits(
            drain_inst.ins, ScopedClock({None: tick_clock.global_clock})
        )
        self.nc.all_engine_barrier()
        assert self.sems is not None
        popped = self.nc._tile_sem_poison_stack.pop()
        assert popped is self._sem_poison
        self.nc.clear_and_free_semaphores(list(self.sems.allocated().values()))


def _build(with_bias):
    nc = bass.Bass()
    AF = mybir.ActivationFunctionType
    ts = bass.ts

    xt = nc.dram_tensor("xt", [BG, P, KO, BG_ROWS], BF16, kind="ExternalInput")
    w = nc.dram_tensor("w", [P, KO, NW], BF16, kind="ExternalInput")
    cell = nc.dram_tensor("cell", [B_LOC, H], F32, kind="ExternalInput")
    if with_bias:
        bias = nc.dram_tensor("bias", [P, NW], F32, kind="ExternalInput")
    h_out = nc.dram_tensor("h_out", [B_LOC, H], F32, kind="ExternalOutput")
    c_out = nc.dram_tensor("c_out", [B_LOC, H], F32, kind="ExternalOutput")

    with _FastTailTileContext(nc) as tc:
        with (
            tc.tile_pool(name="wpool", bufs=1) as wpool,
            tc.tile_pool(name="xpool", bufs=2) as xpool,
            tc.tile_pool(name="cpool", bufs=4) as cpool,
            tc.tile_pool(name="gpool", bufs=3) as gpool,
            tc.tile_pool(name="ppool", bufs=8, space="PSUM") as ppool,
        ):
            bias_t = None
            if with_bias:
                bias_t = wpool.tile([P, NW], F32, tag="bias_t", name="bias_t")
                nc.sync.dma_start(bias_t[:], bias[:])

            def epilogue(ps, ct, rows, uid, splits=1):
                # gates from psum -> cell/hidden update -> DMA out.
                # splits>1 pipelines the serial ACT->DVE->DMA chain in column
                # chunks — used for the last batch tile to shorten the tail.
                if with_bias:
                    zs = []
                    for nn in range(NG):
                        z = gpool.tile([P, H], F32, tag=f"z{nn}", name=f"z{nn}_{uid}")
                        nc.vector.tensor_add(z[:], ps[nn], bias_t[:, ts(nn, H)])
                        zs.append(z)
                else:
                    zs = ps
                w_ = H // splits
                for q in range(splits):
                    cs = slice(q * w_, (q + 1) * w_)
                    i_t = gpool.tile([P, w_], F32, tag="i_t", name=f"i_{uid}_{q}")
                    nc.scalar.activation(i_t[:], zs[0][:, cs], AF.Sigmoid)
                    f_t = gpool.tile([P, w_], F32, tag="f_t", name=f"f_{uid}_{q}")
                    nc.scalar.activation(f_t[:], zs[1][:, cs], AF.Sigmoid)
                    g_t = gpool.tile([P, w_], F32, tag="g_t", name=f"g_{uid}_{q}")
                    nc.scalar.activation(g_t[:], zs[2][:, cs], AF.Tanh)
                    o_t = gpool.tile([P, w_], F32, tag="o_t", name=f"o_{uid}_{q}")
                    nc.scalar.activation(o_t[:], zs[3][:, cs], AF.Sigmoid)

                    fc = gpool.tile([P, w_], F32, tag="fc", name=f"fc_{uid}_{q}")
                    nc.vector.tensor_mul(fc[:], f_t[:], ct[:, cs])
                    ig = gpool.tile([P, w_], F32, tag="ig", name=f"ig_{uid}_{q}")
                    nc.vector.tensor_mul(ig[:], i_t[:], g_t[:])
                    cn = gpool.tile([P, w_], F32, tag="cn", name=f"cn_{uid}_{q}")
                    nc.vector.tensor_add(cn[:], fc[:], ig[:])
                    tn = gpool.tile([P, w_], F32, tag="tn", name=f"tn_{uid}_{q}")
                    nc.scalar.activation(tn[:], cn[:], AF.Tanh)
                    hn = gpool.tile([P, w_], F32, tag="hn", name=f"hn_{uid}_{q}")
                    nc.vector.tensor_mul(hn[:], o_t[:], tn[:])

                    nc.sync.dma_start(c_out[rows, cs], cn[:])
                    nc.sync.dma_start(h_out[rows, cs], hn[:])

            # PE warmup: ~2us of throwaway matmuls so the HAM clock gate
            # opens to 2.4GHz. lhsT rides on the xs0_0 DMA so the warm
            # window starts when the first bytes land and abuts the first
            # real matmul instead of draining early and re-throttling.
            wz = wpool.tile([P, P], BF16, tag="wz", name="wz")
            nc.gpsimd.memset(wz[:], 0.0)

            # Startup DMAs in consumption order: the HW DMA engine delivers
            # ~356GB/s aggregate roughly in trigger order, and each trigger
            # costs ~0.6us on the issuing queue — so interleave each W
            # k-chunk with the slab-0 x chunk the k-loop needs next, as
            # whole chunks (finer splits lose more to trigger serialization
            # than they gain in latency).
            wks, xs0 = [], []
            for k in range(KO):
                x0k = wpool.tile([P, BG_ROWS], BF16, tag=f"xs0_{k}", name=f"xs0_{k}")
                nc.sync.dma_start(x0k[:], xt[0, :, k, :])
                xs0.append(x0k)
                wk = wpool.tile([P, NW], BF16, tag=f"w{k}", name=f"w{k}")
                nc.sync.dma_start(wk[:], w[:, k, :])
                wks.append(wk)
                if k == 0:
                    # Hold the later triggers off the stream for ~1.2us so
                    # the first 640KB (xs0_0+w0, which gate the first real
                    # matmul) get the full HBM rate instead of a 1/N share;
                    # later chunks have ~10us of slack before PE needs them.
                    # (A trigger costs ~0.6us of Sync queue time; these two
                    # move 4KB each, so the bandwidth cost is nil.)
                    for dd in range(2):
                        scr = wpool.tile(
                            [P, 16], BF16, tag=f"scr{dd}", name=f"scr{dd}"
                        )
                        nc.sync.dma_start(scr[:], xt[0, :, 0, :16])

            def w_slice(k, nn):
                return wks[k][:, ts(nn, H)]

            warm_ps = ppool.tile([P, P], F32, tag="ps", name="warm_ps")
            for _ in range(16):
                nc.tensor.matmul(
                    warm_ps[:], xs0[0][:, :P], wz[:], start=True, stop=True
                )

            # Slab 0: k-major over j-pairs so PE consumes each W chunk as it
            # arrives instead of stalling for the whole 4MB of W.
            cts0 = []
            for j in range(BT_PER_BG):
                ct = cpool.tile([P, H], F32, tag="ct", name=f"ct0_{j}")
                nc.sync.dma_start(ct[:], cell[j * P : (j + 1) * P, :])
                cts0.append(ct)
            for jp in (0, 2):
                ps2 = {
                    (j, nn): ppool.tile([P, H], F32, tag="ps", name=f"ps0_{j}_{nn}")
                    for j in (jp, jp + 1)
                    for nn in range(NG)
                }
                for k in range(KO):
                    for j in (jp, jp + 1):
                        lhsT = xs0[k][:, ts(j, P)]
                        for nn in range(NG):
                            nc.tensor.matmul(
                                ps2[(j, nn)],
                                lhsT,
                                w_slice(k, nn),
                                start=(k == 0),
                                stop=(k == KO - 1),
                            )
                for j in (jp, jp + 1):
                    epilogue(
                        [ps2[(j, nn)] for nn in range(NG)],
                        cts0[j],
                        slice(j * P, (j + 1) * P),
                        f"g0_{j}",
                    )

            # Slabs 1..7: j-major, full-rate PE against prefetched slabs.
            for g in range(1, BG):
                xs = xpool.tile([P, KO, BG_ROWS], BF16, tag="xs", name="xs")
                nc.sync.dma_start(xs[:], xt[g])
                for j in range(BT_PER_BG):
                    bt = g * BT_PER_BG + j
                    rows = slice(bt * P, (bt + 1) * P)
                    ct = cpool.tile([P, H], F32, tag="ct", name=f"ct_{bt}")
                    nc.sync.dma_start(ct[:], cell[rows, :])
                    ps = [
                        ppool.tile([P, H], F32, tag="ps", name=f"ps{nn}_{bt}")
                        for nn in range(NG)
                    ]
                    last_bt = g == BG - 1 and j == BT_PER_BG - 1
                    if last_bt:
                        # Bank-by-bank (k-inner) so gates g/i/f are ready
                        # before the final o-bank matmul: the post-MM tail
                        # shrinks to sigmoid(o) -> h -> DMA.
                        for nn in (2, 0, 1, 3):
                            for k in range(KO):
                                nc.tensor.matmul(
                                    ps[nn],
                                    xs[:, k, ts(j, P)],
                                    w_slice(k, nn),
                                    start=(k == 0),
                                    stop=(k == KO - 1),
                                )
                    else:
                        for k in range(KO):
                            lhsT = xs[:, k, ts(j, P)]
                            for nn in range(NG):
                                nc.tensor.matmul(
                                    ps[nn],
                                    lhsT,
                                    w_slice(k, nn),
                                    start=(k == 0),
                                    stop=(k == KO - 1),
                                )
                    epilogue(ps, ct, rows, f"g{g}_{j}")

    _split_multi_waits(nc)
    return nc


def kernel(input, cell_state, Wi, bi, Wf, bf, Wg, bg, Wo, bo):
    global LAST_RESULTS

    x = np.asarray(input, dtype=np.float32)
    cell = np.ascontiguousarray(np.asarray(cell_state, dtype=np.float32))
    Wcat = np.concatenate(
        [np.asarray(m, dtype=np.float32) for m in (Wi, Wf, Wg, Wo)], axis=1
    )  # [D, 4H]
    bcat = np.concatenate(
        [np.asarray(v, dtype=np.float32) for v in (bi, bf, bg, bo)]
    )  # [4H]
    with_bias = bool(np.any(bcat))

    # W -> [p, ko, n] bf16, contiguous per partition.
    w_dev = np.ascontiguousarray(
        Wcat.astype(ml_dtypes.bfloat16).reshape(KO, P, NW).transpose(1, 0, 2)
    )

    in_maps = []
    for c in range(N_CORES):
        xc = x[c * B_LOC : (c + 1) * B_LOC]  # [4096, 1024]
        # -> [bg, p, ko, b] so each 512-row slab DMA is contiguous/partition.
        xt_c = np.ascontiguousarray(
            xc.astype(ml_dtypes.bfloat16)
            .reshape(BG, BG_ROWS, KO, P)
            .transpose(0, 3, 2, 1)
        )
        m = {
            "xt": xt_c,
            "w": w_dev,
            "cell": cell[c * B_LOC : (c + 1) * B_LOC],
        }
        if with_bias:
            m["bias"] = np.ascontiguousarray(
                np.broadcast_to(bcat[None, :], (P, NW)).astype(np.float32)
            )
        in_maps.append(m)

    key = with_bias
    if key not in _CACHED:
        _CACHED[key] = _build(with_bias)
    nc = _CACHED[key]

    trace = os.environ.get("KERNEL_TRACE", "0") == "1"
    res = run_bass_kernel_spmd(nc, in_maps, list(range(N_CORES)), trace=trace)
    LAST_RESULTS = res

    h = np.concatenate([res.results[c]["h_out"] for c in range(N_CORES)], axis=0)
    c_ = np.concatenate([res.results[c]["c_out"] for c in range(N_CORES)], axis=0)
    return h, c_



# revision 2
# speedup vs baseline: 1.2517x; 1.2517x over previous
"""CustomLSTM cell (4 gated projections + cell update) on 8 TRN2 NeuronCores.

Data-parallel over the batch dim: each core processes B/8 = 4096 rows.
Per core the 4 gate projections split by precision: the i/f gates run as
fp8-e4m3 DoubleRow matmuls (2 contraction rows per PE cell -> 2x rate;
quantization error is damped by sigmoid's <=0.25 slope before it reaches
c_t/h_t), while the g (tanh) and o gates — whose errors pass through
near-unit derivatives — stay bf16.  Host-side prep lays x out transposed
([p, ko, b] per 512-row batch group) in both bf16 and scaled e4m3
(x*16, W*512; the 2^-13 dequant rides the activation's scale operand),
so every DMA is contiguous per partition.  PSUM accumulation stays f32.

Self-contained: shapes/sharding hardcoded for
input [32768, 1024], cell_state [32768, 512], W* [1024, 512].
"""

import os

import numpy as np
import ml_dtypes

import bass_rust
import concourse.bass as bass
import concourse.mybir as mybir
import concourse.tile as tile
from concourse.bass_utils import run_bass_kernel_spmd

N_CORES = 8
B = 32768
D = 1024
H = 512
P = 128
B_LOC = B // N_CORES        # 4096 rows per core
KO = D // P                 # 8 k-subtiles
KC = KO // 2                # 4 fp8 DoubleRow chunks (K=256 each)
NW = 4 * H                  # 2048 concatenated gate dim
NH = 2 * H                  # 1024: one precision-pair of gates
BG_ROWS = 512               # batch rows per x slab
BG = B_LOC // BG_ROWS       # 8 slabs per core
BT_PER_BG = BG_ROWS // P    # 4 batch tiles per slab

SX = 16.0                   # x fp8 scale
SW = 512.0                  # W fp8 scale
DEQ = 1.0 / (SX * SW)       # 2^-13, exact in f32

BF16 = mybir.dt.bfloat16
F8 = mybir.dt.float8e4
F32 = mybir.dt.float32
DR = mybir.MatmulPerfMode.DoubleRow

# Filled by the last kernel() call: BassKernelResults (exec_time_ns etc).
LAST_RESULTS = None
_CACHED = {}


def _split_multi_waits(nc):
    """Legalize for a walrus build that accepts one sync-wait per instruction.

    Tile's wait assignment attaches every needed sem wait to the consuming
    instruction; this backend rejects >1 ("Too many sync wait commands").
    Move all but the last wait onto dedicated NoOps inserted just before the
    instruction on the same engine queue — sequential waits on one engine are
    equivalent to a single multi-wait instruction for monotone sem waits.
    """
    n = 0
    for f in nc.m.functions:
        for blk in f.blocks:
            insts = blk.instructions
            if not any(
                i.sync_info is not None and len(i.sync_info.on_wait) > 1
                for i in insts
            ):
                continue
            out = []
            for inst in insts:
                si = inst.sync_info
                if si is not None and len(si.on_wait) > 1:
                    waits = list(si.on_wait)
                    for w in waits[:-1]:
                        nop = mybir.InstNoOp(name=f"waitsplit_{n}", ins=[], outs=[])
                        n += 1
                        nop.engine = inst.engine
                        nop.sync_info = bass_rust.SyncInfo(on_wait=[w], on_update=[])
                        out.append(nop)
                    inst.sync_info = bass_rust.SyncInfo(
                        on_wait=[waits[-1]], on_update=list(si.on_update)
                    )
                out.append(inst)
            blk.instructions = out


class _FastTailTileContext(tile.TileContext):
    """Drop the second tail all-engine barrier.

    The stock tail is [drain+waits][barrier][sem/queue reset][barrier]; the
    final barrier only isolates the gpsimd-side reset from code that would
    follow it — nothing follows it here, and NRT waits for every engine
    stream (including gpsimd's reset) to halt before completion, so engines
    can end right after the first barrier. Saves ~4-6us of EVSEM ring.
    """

    def _drain_and_barrier(self, tick_clock, wait_clock):
        from concourse.vector_clock import ScopedClock

        drain_inst = self.nc.sync.drain()
        wait_clock.add_sem_waits(
            drain_inst.ins, ScopedClock({None: tick_clock.global_clock})
        )
        self.nc.all_engine_barrier()
        assert self.sems is not None
        popped = self.nc._tile_sem_poison_stack.pop()
        assert popped is self._sem_poison
        self.nc.clear_and_free_semaphores(list(self.sems.allocated().values()))


def _build(with_bias):
    nc = bass.Bass()
    AF = mybir.ActivationFunctionType
    ts = bass.ts

    xt = nc.dram_tensor("xt", [BG, P, KO, BG_ROWS], BF16, kind="ExternalInput")
    x8t = nc.dram_tensor("x8t", [BG, P, KO, BG_ROWS], F8, kind="ExternalInput")
    w = nc.dram_tensor("w", [P, KO, NH], BF16, kind="ExternalInput")
    w8 = nc.dram_tensor("w8", [P, KO, NH], F8, kind="ExternalInput")
    cell = nc.dram_tensor("cell", [B_LOC, H], F32, kind="ExternalInput")
    if with_bias:
        bias = nc.dram_tensor("bias", [P, NW], F32, kind="ExternalInput")
    h_out = nc.dram_tensor("h_out", [B_LOC, H], F32, kind="ExternalOutput")
    c_out = nc.dram_tensor("c_out", [B_LOC, H], F32, kind="ExternalOutput")

    with _FastTailTileContext(nc) as tc:
        with (
            tc.tile_pool(name="wpool", bufs=1) as wpool,
            tc.tile_pool(name="xpool", bufs=2) as xpool,
            tc.tile_pool(name="x8pool", bufs=2) as x8pool,
            tc.tile_pool(name="cpool", bufs=4) as cpool,
            tc.tile_pool(name="gpool", bufs=3) as gpool,
            tc.tile_pool(name="ppool", bufs=8, space="PSUM") as ppool,
        ):
            bias_t = None
            if with_bias:
                bias_t = wpool.tile([P, NW], F32, tag="bias_t", name="bias_t")
                nc.sync.dma_start(bias_t[:], bias[:])

            def epilogue(ps, ct, rows, uid, splits=1):
                # ps = [i, f, g, o] psum banks; i/f hold 8192*z (fp8 scales),
                # dequantized for free via the activation scale operand.
                # gates from psum -> cell/hidden update -> DMA out.
                if with_bias:
                    # bias varies along the free dim so it can't ride the
                    # activation's per-partition bias operand: materialize
                    # z = deq*ps + b via a scaled Copy then a vector add.
                    zs = []
                    for nn in range(4):
                        sc = DEQ if nn < 2 else 1.0
                        zq = gpool.tile([P, H], F32, tag=f"zq{nn}", name=f"zq{nn}_{uid}")
                        nc.scalar.activation(zq[:], ps[nn], AF.Copy, scale=sc)
                        z = gpool.tile([P, H], F32, tag=f"z{nn}", name=f"z{nn}_{uid}")
                        nc.vector.tensor_add(z[:], zq[:], bias_t[:, ts(nn, H)])
                        zs.append(z)
                    scales = [1.0, 1.0, 1.0, 1.0]
                else:
                    zs = ps
                    scales = [DEQ, DEQ, 1.0, 1.0]
                w_ = H // splits
                for q in range(splits):
                    cs = slice(q * w_, (q + 1) * w_)
                    i_t = gpool.tile([P, w_], F32, tag="i_t", name=f"i_{uid}_{q}")
                    nc.scalar.activation(i_t[:], zs[0][:, cs], AF.Sigmoid, scale=scales[0])
                    f_t = gpool.tile([P, w_], F32, tag="f_t", name=f"f_{uid}_{q}")
                    nc.scalar.activation(f_t[:], zs[1][:, cs], AF.Sigmoid, scale=scales[1])
                    g_t = gpool.tile([P, w_], F32, tag="g_t", name=f"g_{uid}_{q}")
                    nc.scalar.activation(g_t[:], zs[2][:, cs], AF.Tanh, scale=scales[2])
                    o_t = gpool.tile([P, w_], F32, tag="o_t", name=f"o_{uid}_{q}")
                    nc.scalar.activation(o_t[:], zs[3][:, cs], AF.Sigmoid, scale=scales[3])

                    fc = gpool.tile([P, w_], F32, tag="fc", name=f"fc_{uid}_{q}")
                    nc.vector.tensor_mul(fc[:], f_t[:], ct[:, cs])
                    ig = gpool.tile([P, w_], F32, tag="ig", name=f"ig_{uid}_{q}")
                    nc.vector.tensor_mul(ig[:], i_t[:], g_t[:])
                    cn = gpool.tile([P, w_], F32, tag="cn", name=f"cn_{uid}_{q}")
                    nc.vector.tensor_add(cn[:], fc[:], ig[:])
                    tn = gpool.tile([P, w_], F32, tag="tn", name=f"tn_{uid}_{q}")
                    nc.scalar.activation(tn[:], cn[:], AF.Tanh)
                    hn = gpool.tile([P, w_], F32, tag="hn", name=f"hn_{uid}_{q}")
                    nc.vector.tensor_mul(hn[:], o_t[:], tn[:])

                    nc.sync.dma_start(c_out[rows, cs], cn[:])
                    nc.sync.dma_start(h_out[rows, cs], hn[:])

            # PE warmup: ~2us of throwaway matmuls so the HAM clock gate
            # opens to 2.4GHz. lhsT rides on the xs0_0 DMA so the warm
            # window starts when the first bytes land and abuts the first
            # real matmul instead of draining early and re-throttling.
            wz = wpool.tile([P, P], BF16, tag="wz", name="wz")
            nc.gpsimd.memset(wz[:], 0.0)

            # Startup DMAs in consumption order: the HW DMA engine delivers
            # ~356GB/s aggregate roughly in trigger order, and each trigger
            # costs ~0.6us on the issuing queue — so interleave each W
            # k-chunk with the slab-0 x chunk the k-loop needs next, as
            # whole chunks (finer splits lose more to trigger serialization
            # than they gain in latency).  bf16 g/o stream first (they run
            # first), then the fp8 i/f chunk pairs.
            wks, xs0 = [], []
            for k in range(KO):
                x0k = wpool.tile([P, BG_ROWS], BF16, tag=f"xs0_{k}", name=f"xs0_{k}")
                nc.sync.dma_start(x0k[:], xt[0, :, k, :])
                xs0.append(x0k)
                wk = wpool.tile([P, NH], BF16, tag=f"w{k}", name=f"w{k}")
                nc.sync.dma_start(wk[:], w[:, k, :])
                wks.append(wk)
                if k == 0:
                    # Hold the later triggers off the stream for ~1.2us so
                    # the first 384KB (xs0_0+w0, which gate the first real
                    # matmul) get the full HBM rate instead of a 1/N share.
                    for dd in range(2):
                        scr = wpool.tile(
                            [P, 16], BF16, tag=f"scr{dd}", name=f"scr{dd}"
                        )
                        nc.sync.dma_start(scr[:], xt[0, :, 0, :16])
            x8s0, w8ks = [], []
            for c in range(KC):
                x8c = wpool.tile([P, 2, BG_ROWS], F8, tag=f"x8s0_{c}", name=f"x8s0_{c}")
                nc.sync.dma_start(x8c[:], x8t[0, :, 2 * c : 2 * c + 2, :])
                x8s0.append(x8c)
                w8c = wpool.tile([P, 2, NH], F8, tag=f"w8_{c}", name=f"w8_{c}")
                nc.sync.dma_start(w8c[:], w8[:, 2 * c : 2 * c + 2, :])
                w8ks.append(w8c)

            warm_ps = ppool.tile([P, P], F32, tag="ps", name="warm_ps")
            for _ in range(16):
                nc.tensor.matmul(
                    warm_ps[:], xs0[0][:, :P], wz[:], start=True, stop=True
                )

            # Slab 0: k-major over j-pairs so PE consumes each W chunk as it
            # arrives instead of stalling for the whole 3MB of W: bf16 g/o
            # ride the xs0/w stream, fp8 i/f ride the x8/w8 stream after it.
            cts0 = []
            for j in range(BT_PER_BG):
                ct = cpool.tile([P, H], F32, tag="ct", name=f"ct0_{j}")
                nc.sync.dma_start(ct[:], cell[j * P : (j + 1) * P, :])
                cts0.append(ct)
            for jp in (0, 2):
                ps2 = {
                    (j, nn): ppool.tile([P, H], F32, tag="ps", name=f"ps0_{j}_{nn}")
                    for j in (jp, jp + 1)
                    for nn in range(4)
                }
                for k in range(KO):
                    for j in (jp, jp + 1):
                        lhsT = xs0[k][:, ts(j, P)]
                        for nn in (2, 3):  # g, o in bf16
                            nc.tensor.matmul(
                                ps2[(j, nn)],
                                lhsT,
                                wks[k][:, ts(nn - 2, H)],
                                start=(k == 0),
                                stop=(k == KO - 1),
                            )
                for c in range(KC):
                    for j in (jp, jp + 1):
                        lhsT8 = x8s0[c][:, :, ts(j, P)]
                        for nn in (0, 1):  # i, f in fp8 DoubleRow
                            nc.tensor.matmul(
                                ps2[(j, nn)],
                                lhsT8,
                                w8ks[c][:, :, ts(nn, H)],
                                start=(c == 0),
                                stop=(c == KC - 1),
                                perf_mode=DR,
                            )
                for j in (jp, jp + 1):
                    epilogue(
                        [ps2[(j, nn)] for nn in range(4)],
                        cts0[j],
                        slice(j * P, (j + 1) * P),
                        f"g0_{j}",
                    )

            # Slabs 1..7: j-major, full-rate PE against prefetched slabs.
            for g in range(1, BG):
                xs = xpool.tile([P, KO, BG_ROWS], BF16, tag="xs", name="xs")
                nc.sync.dma_start(xs[:], xt[g])
                x8s = x8pool.tile([P, KO, BG_ROWS], F8, tag="x8s", name="x8s")
                nc.sync.dma_start(x8s[:], x8t[g])
                for j in range(BT_PER_BG):
                    bt = g * BT_PER_BG + j
                    rows = slice(bt * P, (bt + 1) * P)
                    ct = cpool.tile([P, H], F32, tag="ct", name=f"ct_{bt}")
                    nc.sync.dma_start(ct[:], cell[rows, :])
                    ps = [
                        ppool.tile([P, H], F32, tag="ps", name=f"ps{nn}_{bt}")
                        for nn in range(4)
                    ]
                    last_bt = g == BG - 1 and j == BT_PER_BG - 1
                    # fp8 i/f first (their epilogue work is mid-chain), then
                    # bf16 g before o: for the last tile this leaves only
                    # sigmoid(o) -> h -> DMA after the final matmul.
                    for c in range(KC):
                        lhsT8 = x8s[:, 2 * c : 2 * c + 2, ts(j, P)]
                        for nn in (0, 1):
                            nc.tensor.matmul(
                                ps[nn],
                                lhsT8,
                                w8ks[c][:, :, ts(nn, H)],
                                start=(c == 0),
                                stop=(c == KC - 1),
                                perf_mode=DR,
                            )
                    if last_bt:
                        for nn in (2, 3):
                            for k in range(KO):
                                nc.tensor.matmul(
                                    ps[nn],
                                    xs[:, k, ts(j, P)],
                                    wks[k][:, ts(nn - 2, H)],
                                    start=(k == 0),
                                    stop=(k == KO - 1),
                                )
                    else:
                        for k in range(KO):
                            lhsT = xs[:, k, ts(j, P)]
                            for nn in (2, 3):
                                nc.tensor.matmul(
                                    ps[nn],
                                    lhsT,
                                    wks[k][:, ts(nn - 2, H)],
                                    start=(k == 0),
                                    stop=(k == KO - 1),
                                )
                    epilogue(ps, ct, rows, f"g{g}_{j}")

    _split_multi_waits(nc)
    return nc


def kernel(input, cell_state, Wi, bi, Wf, bf, Wg, bg, Wo, bo):
    global LAST_RESULTS

    x = np.asarray(input, dtype=np.float32)
    cell = np.ascontiguousarray(np.asarray(cell_state, dtype=np.float32))
    Wif = np.concatenate(
        [np.asarray(m, dtype=np.float32) for m in (Wi, Wf)], axis=1
    )  # [D, 2H]
    Wgo = np.concatenate(
        [np.asarray(m, dtype=np.float32) for m in (Wg, Wo)], axis=1
    )  # [D, 2H]
    bcat = np.concatenate(
        [np.asarray(v, dtype=np.float32) for v in (bi, bf, bg, bo)]
    )  # [4H]
    with_bias = bool(np.any(bcat))

    # W -> [p, ko, n], contiguous per partition; k = 128*ko + p.
    w_dev = np.ascontiguousarray(
        Wgo.astype(ml_dtypes.bfloat16).reshape(KO, P, NH).transpose(1, 0, 2)
    )
    w8_dev = np.ascontiguousarray(
        np.clip(Wif * SW, -240.0, 240.0)
        .astype(ml_dtypes.float8_e4m3)
        .reshape(KO, P, NH)
        .transpose(1, 0, 2)
    )

    in_maps = []
    for c in range(N_CORES):
        xc = x[c * B_LOC : (c + 1) * B_LOC]  # [4096, 1024]
        # -> [bg, p, ko, b] so each 512-row slab DMA is contiguous/partition.
        xt_c = np.ascontiguousarray(
            xc.astype(ml_dtypes.bfloat16)
            .reshape(BG, BG_ROWS, KO, P)
            .transpose(0, 3, 2, 1)
        )
        x8t_c = np.ascontiguousarray(
            np.clip(xc * SX, -240.0, 240.0)
            .astype(ml_dtypes.float8_e4m3)
            .reshape(BG, BG_ROWS, KO, P)
            .transpose(0, 3, 2, 1)
        )
        m = {
            "xt": xt_c,
            "x8t": x8t_c,
            "w": w_dev,
            "w8": w8_dev,
            "cell": cell[c * B_LOC : (c + 1) * B_LOC],
        }
        if with_bias:
            m["bias"] = np.ascontiguousarray(
                np.broadcast_to(bcat[None, :], (P, NW)).astype(np.float32)
            )
        in_maps.append(m)

    key = with_bias
    if key not in _CACHED:
        _CACHED[key] = _build(with_bias)
    nc = _CACHED[key]

    trace = os.environ.get("KERNEL_TRACE", "0") == "1"
    res = run_bass_kernel_spmd(nc, in_maps, list(range(N_CORES)), trace=trace)
    LAST_RESULTS = res

    h = np.concatenate([res.results[c]["h_out"] for c in range(N_CORES)], axis=0)
    c_ = np.concatenate([res.results[c]["c_out"] for c in range(N_CORES)], axis=0)
    return h, c_


# revision 4
# speedup vs baseline: 1.3470x; 1.0761x over previous
"""CustomLSTM cell (4 gated projections + cell update) on 8 TRN2 NeuronCores.

Data-parallel over the batch dim: each core processes B/8 = 4096 rows.
Per core the 4 gate projections split by precision: the i/f gates run as
fp8-e4m3 DoubleRow matmuls (2 contraction rows per PE cell -> 2x rate;
quantization error is damped by sigmoid's <=0.25 slope before it reaches
c_t/h_t), the o gate runs half-K fp8 / half-K bf16 (its error reaches h_t
through one sigmoid), and g (tanh, near-unit slope into c_t) stays bf16.
The bf16 o-half is scaled by 8192 host-side (exact exponent shift) so both
halves share one PSUM accumulation; the 2^-13 dequant rides the sigmoid's
scale operand.  Host-side prep lays x out transposed ([p, ko, b] per
512-row slab) in both bf16 and scaled e4m3; every DMA is contiguous per
partition.  PSUM accumulation stays f32.

Self-contained: shapes/sharding hardcoded for
input [32768, 1024], cell_state [32768, 512], W* [1024, 512].
"""

import os

import numpy as np
import ml_dtypes

import bass_rust
import concourse.bass as bass
import concourse.mybir as mybir
import concourse.tile as tile
from concourse.bass_utils import run_bass_kernel_spmd

N_CORES = 8
B = 32768
D = 1024
H = 512
P = 128
B_LOC = B // N_CORES        # 4096 rows per core
KO = D // P                 # 8 k-subtiles
KC = KO // 2                # 4 fp8 DoubleRow chunks (K=256 each)
KH = KO // 2                # 4: k-subtiles in each half of K
NW = 4 * H                  # 2048 concatenated gate dim
NH = 2 * H                  # 1024: i|f pair width
BG_ROWS = 512               # batch rows per x slab
BG = B_LOC // BG_ROWS       # 8 slabs per core
BT_PER_BG = BG_ROWS // P    # 4 batch tiles per slab

SX = 16.0                   # x fp8 scale
SW = 512.0                  # W fp8 scale
SO = SX * SW                # 8192: bf16 o-half pre-scale (exact in bf16)
DEQ = 1.0 / (SX * SW)       # 2^-13, exact in f32

BF16 = mybir.dt.bfloat16
F8 = mybir.dt.float8e4
F32 = mybir.dt.float32
DR = mybir.MatmulPerfMode.DoubleRow

# Filled by the last kernel() call: BassKernelResults (exec_time_ns etc).
LAST_RESULTS = None
_CACHED = {}


def _split_multi_waits(nc):
    """Legalize for a walrus build that accepts one sync-wait per instruction.

    Tile's wait assignment attaches every needed sem wait to the consuming
    instruction; this backend rejects >1 ("Too many sync wait commands").
    Move all but the last wait onto dedicated NoOps inserted just before the
    instruction on the same engine queue — sequential waits on one engine are
    equivalent to a single multi-wait instruction for monotone sem waits.
    """
    n = 0
    for f in nc.m.functions:
        for blk in f.blocks:
            insts = blk.instructions
            if not any(
                i.sync_info is not None and len(i.sync_info.on_wait) > 1
                for i in insts
            ):
                continue
            out = []
            for inst in insts:
                si = inst.sync_info
                if si is not None and len(si.on_wait) > 1:
                    waits = list(si.on_wait)
                    for w in waits[:-1]:
                        nop = mybir.InstNoOp(name=f"waitsplit_{n}", ins=[], outs=[])
                        n += 1
                        nop.engine = inst.engine
                        nop.sync_info = bass_rust.SyncInfo(on_wait=[w], on_update=[])
                        out.append(nop)
                    inst.sync_info = bass_rust.SyncInfo(
                        on_wait=[waits[-1]], on_update=list(si.on_update)
                    )
                out.append(inst)
            blk.instructions = out


class _FastTailTileContext(tile.TileContext):
    """Drop the second tail all-engine barrier.

    The stock tail is [drain+waits][barrier][sem/queue reset][barrier]; the
    final barrier only isolates the gpsimd-side reset from code that would
    follow it — nothing follows it here, and NRT waits for every engine
    stream (including gpsimd's reset) to halt before completion, so engines
    can end right after the first barrier. Saves ~4-6us of EVSEM ring.
    """

    def _drain_and_barrier(self, tick_clock, wait_clock):
        from concourse.vector_clock import ScopedClock

        drain_inst = self.nc.sync.drain()
        wait_clock.add_sem_waits(
            drain_inst.ins, ScopedClock({None: tick_clock.global_clock})
        )
        self.nc.all_engine_barrier()
        assert self.sems is not None
        popped = self.nc._tile_sem_poison_stack.pop()
        assert popped is self._sem_poison
        self.nc.clear_and_free_semaphores(list(self.sems.allocated().values()))


def _build(with_bias):
    nc = bass.Bass()
    AF = mybir.ActivationFunctionType
    ts = bass.ts

    xt = nc.dram_tensor("xt", [BG, P, KO, BG_ROWS], BF16, kind="ExternalInput")
    x8t = nc.dram_tensor("x8t", [BG, P, KO, BG_ROWS], F8, kind="ExternalInput")
    # bf16 weights: k-chunks 0..3 carry [g | 8192*o], chunks 4..7 carry g only.
    w1 = nc.dram_tensor("w1", [P, KH, NH], BF16, kind="ExternalInput")
    w2 = nc.dram_tensor("w2", [P, KH, H], BF16, kind="ExternalInput")
    w8 = nc.dram_tensor("w8", [P, KO, NH], F8, kind="ExternalInput")    # i|f
    w8o = nc.dram_tensor("w8o", [P, KH, H], F8, kind="ExternalInput")   # o, k 512..
    cell = nc.dram_tensor("cell", [B_LOC, H], BF16, kind="ExternalInput")
    if with_bias:
        bias = nc.dram_tensor("bias", [P, NW], F32, kind="ExternalInput")
    h_out = nc.dram_tensor("h_out", [B_LOC, H], F32, kind="ExternalOutput")
    c_out = nc.dram_tensor("c_out", [B_LOC, H], F32, kind="ExternalOutput")

    with _FastTailTileContext(nc) as tc:
        with (
            tc.tile_pool(name="wpool", bufs=1) as wpool,
            tc.tile_pool(name="xpool", bufs=2) as xpool,
            tc.tile_pool(name="x8pool", bufs=2) as x8pool,
            tc.tile_pool(name="cpool", bufs=4) as cpool,
            tc.tile_pool(name="gpool", bufs=3) as gpool,
            tc.tile_pool(name="ppool", bufs=8, space="PSUM") as ppool,
        ):
            bias_t = None
            if with_bias:
                bias_t = wpool.tile([P, NW], F32, tag="bias_t", name="bias_t")
                nc.sync.dma_start(bias_t[:], bias[:])

            def epilogue(ps, ct, rows, uid, splits=1):
                # ps = [i, f, g, o]; i/f/o psums hold 8192*z (fp8 scaling),
                # dequantized for free via the activation scale operand.
                if with_bias:
                    # bias varies along the free dim so it can't ride the
                    # activation's per-partition bias operand: materialize
                    # z = deq*ps + b via a scaled Copy then a vector add.
                    zs = []
                    for nn in range(4):
                        sc = 1.0 if nn == 2 else DEQ
                        zq = gpool.tile([P, H], F32, tag=f"zq{nn}", name=f"zq{nn}_{uid}")
                        nc.scalar.activation(zq[:], ps[nn], AF.Copy, scale=sc)
                        z = gpool.tile([P, H], F32, tag=f"z{nn}", name=f"z{nn}_{uid}")
                        nc.vector.tensor_add(z[:], zq[:], bias_t[:, ts(nn, H)])
                        zs.append(z)
                    scales = [1.0, 1.0, 1.0, 1.0]
                else:
                    zs = ps
                    scales = [DEQ, DEQ, 1.0, DEQ]
                w_ = H // splits
                for q in range(splits):
                    cs = slice(q * w_, (q + 1) * w_)
                    i_t = gpool.tile([P, w_], F32, tag="i_t", name=f"i_{uid}_{q}")
                    nc.scalar.activation(i_t[:], zs[0][:, cs], AF.Sigmoid, scale=scales[0])
                    f_t = gpool.tile([P, w_], F32, tag="f_t", name=f"f_{uid}_{q}")
                    nc.scalar.activation(f_t[:], zs[1][:, cs], AF.Sigmoid, scale=scales[1])
                    g_t = gpool.tile([P, w_], F32, tag="g_t", name=f"g_{uid}_{q}")
                    nc.scalar.activation(g_t[:], zs[2][:, cs], AF.Tanh, scale=scales[2])
                    o_t = gpool.tile([P, w_], F32, tag="o_t", name=f"o_{uid}_{q}")
                    nc.scalar.activation(o_t[:], zs[3][:, cs], AF.Sigmoid, scale=scales[3])

                    fc = gpool.tile([P, w_], F32, tag="fc", name=f"fc_{uid}_{q}")
                    nc.vector.tensor_mul(fc[:], f_t[:], ct[:, cs])
                    ig = gpool.tile([P, w_], F32, tag="ig", name=f"ig_{uid}_{q}")
                    nc.vector.tensor_mul(ig[:], i_t[:], g_t[:])
                    cn = gpool.tile([P, w_], F32, tag="cn", name=f"cn_{uid}_{q}")
                    nc.vector.tensor_add(cn[:], fc[:], ig[:])
                    tn = gpool.tile([P, w_], F32, tag="tn", name=f"tn_{uid}_{q}")
                    nc.scalar.activation(tn[:], cn[:], AF.Tanh)
                    hn = gpool.tile([P, w_], F32, tag="hn", name=f"hn_{uid}_{q}")
                    nc.vector.tensor_mul(hn[:], o_t[:], tn[:])

                    nc.sync.dma_start(c_out[rows, cs], cn[:])
                    nc.sync.dma_start(h_out[rows, cs], hn[:])

            # PE warmup: ~2us of throwaway matmuls so the HAM clock gate
            # opens to 2.4GHz.  The warm lhsT is a tiny dedicated 32KB DMA
            # triggered FIRST — DMA triggers serialize at ~0.65us each on
            # the Sync queue, so the warm window opens ~2us before the
            # first full x chunk lands and abuts the first real matmul.
            wz = wpool.tile([P, P], BF16, tag="wz", name="wz")
            nc.gpsimd.memset(wz[:], 0.0)
            xwarm = wpool.tile([P, P], BF16, tag="xwarm", name="xwarm")
            nc.sync.dma_start(xwarm[:], xt[0, :, 0, :P])

            # Startup DMAs in exact consumption order (trigger serialization
            # makes order = arrival order): bf16 g/o-lo stream, fp8 i/f/o-hi
            # stream, first cells, slab-1 prefetch, remaining cells.
            wks, xs0 = [], []
            for k in range(KO):
                x0k = wpool.tile([P, BG_ROWS], BF16, tag=f"xs0_{k}", name=f"xs0_{k}")
                nc.sync.dma_start(x0k[:], xt[0, :, k, :])
                xs0.append(x0k)
                if k < KH:
                    wk = wpool.tile([P, NH], BF16, tag=f"w{k}", name=f"w{k}")
                    nc.sync.dma_start(wk[:], w1[:, k, :])
                else:
                    wk = wpool.tile([P, H], BF16, tag=f"w{k}", name=f"w{k}")
                    nc.sync.dma_start(wk[:], w2[:, k - KH, :])
                wks.append(wk)
            x8s0, w8ks = [], []
            for c in range(KC):
                x8c = wpool.tile([P, 2, BG_ROWS], F8, tag=f"x8s0_{c}", name=f"x8s0_{c}")
                nc.sync.dma_start(x8c[:], x8t[0, :, 2 * c : 2 * c + 2, :])
                x8s0.append(x8c)
                w8c = wpool.tile([P, 2, NH], F8, tag=f"w8_{c}", name=f"w8_{c}")
                nc.sync.dma_start(w8c[:], w8[:, 2 * c : 2 * c + 2, :])
                w8ks.append(w8c)
            w8ot = wpool.tile([P, KH, H], F8, tag="w8o", name="w8o")
            nc.sync.dma_start(w8ot[:], w8o[:])

            warm_ps = ppool.tile([P, P], F32, tag="ps", name="warm_ps")
            for _ in range(16):
                nc.tensor.matmul(
                    warm_ps[:], xwarm[:], wz[:], start=True, stop=True
                )

            def mm_tile(ps, xs_get, x8s_get, last=False):
                """All matmuls for one batch tile into psum banks ps[i,f,g,o].

                xs_get(k) -> bf16 lhsT [P, P] for k-subtile k;
                x8s_get(c) -> fp8 lhsT [P, 2, P] for DoubleRow chunk c.
                Normal order: bf16 (g + o-lo) first, fp8 chunks after —
                matches the slab-0 DMA stream.  last=True orders o's final
                matmul last so the tail is just sigmoid(o) -> h -> DMA.
                """
                def bf16_part(o_lo):
                    for k in range(KO):
                        lhsT = xs_get(k)
                        nc.tensor.matmul(
                            ps[2], lhsT, wks[k][:, :H],
                            start=(k == 0), stop=(k == KO - 1),
                        )
                        if o_lo and k < KH:
                            nc.tensor.matmul(
                                ps[3], lhsT, wks[k][:, H:NH],
                                start=(k == 0), stop=False,
                            )

                def fp8_part(o_hi_stop):
                    for c in range(KC):
                        lhsT8 = x8s_get(c)
                        for nn in (0, 1):
                            nc.tensor.matmul(
                                ps[nn], lhsT8, w8ks[c][:, :, ts(nn, H)],
                                start=(c == 0), stop=(c == KC - 1),
                                perf_mode=DR,
                            )
                        if c >= 2:
                            cp = c - 2
                            nc.tensor.matmul(
                                ps[3], lhsT8,
                                w8ot[:, 2 * cp : 2 * cp + 2, :],
                                start=False,
                                stop=(o_hi_stop and cp == 1),
                                perf_mode=DR,
                            )

                if not last:
                    bf16_part(o_lo=True)
                    fp8_part(o_hi_stop=True)
                else:
                    # Bank-by-bank (i, f, g, then o) so i/f/g's epilogue
                    # work overlaps o's matmuls and only sigmoid(o) -> h
                    # -> DMA trails the final matmul.
                    for nn in (0, 1):
                        for c in range(KC):
                            nc.tensor.matmul(
                                ps[nn], x8s_get(c), w8ks[c][:, :, ts(nn, H)],
                                start=(c == 0), stop=(c == KC - 1),
                                perf_mode=DR,
                            )
                    bf16_part(o_lo=False)
                    for k in range(KH):
                        nc.tensor.matmul(
                            ps[3], xs_get(k), wks[k][:, H:NH],
                            start=(k == 0), stop=False,
                        )
                    for cp in (0, 1):
                        nc.tensor.matmul(
                            ps[3], x8s_get(cp + 2),
                            w8ot[:, 2 * cp : 2 * cp + 2, :],
                            start=False, stop=(cp == 1),
                            perf_mode=DR,
                        )

            # Slab 0: k-major over j-pairs so PE consumes each chunk as it
            # arrives instead of stalling for the whole weight set.
            cts0 = []
            for j in range(BT_PER_BG):
                ct = cpool.tile([P, H], BF16, tag="ct", name=f"ct0_{j}")
                cts0.append(ct)
            nc.sync.dma_start(cts0[0][:], cell[0:P, :])
            nc.sync.dma_start(cts0[1][:], cell[P : 2 * P, :])
            # Slab-1 prefetch rides here: its trigger must precede the
            # remaining cell loads or PE stalls at the slab seam.
            xs_n = xpool.tile([P, KO, BG_ROWS], BF16, tag="xs", name="xs_1")
            nc.sync.dma_start(xs_n[:], xt[1])
            x8s_n = x8pool.tile([P, KO, BG_ROWS], F8, tag="x8s", name="x8s_1")
            nc.sync.dma_start(x8s_n[:], x8t[1])
            nc.sync.dma_start(cts0[2][:], cell[2 * P : 3 * P, :])
            nc.sync.dma_start(cts0[3][:], cell[3 * P : 4 * P, :])

            for jp in (0, 2):
                ps2 = {
                    (j, nn): ppool.tile([P, H], F32, tag="ps", name=f"ps0_{j}_{nn}")
                    for j in (jp, jp + 1)
                    for nn in range(4)
                }
                for k in range(KO):
                    for j in (jp, jp + 1):
                        lhsT = xs0[k][:, ts(j, P)]
                        nc.tensor.matmul(
                            ps2[(j, 2)], lhsT, wks[k][:, :H],
                            start=(k == 0), stop=(k == KO - 1),
                        )
                        if k < KH:
                            nc.tensor.matmul(
                                ps2[(j, 3)], lhsT, wks[k][:, H:NH],
                                start=(k == 0), stop=False,
                            )
                for c in range(KC):
                    for j in (jp, jp + 1):
                        lhsT8 = x8s0[c][:, :, ts(j, P)]
                        for nn in (0, 1):
                            nc.tensor.matmul(
                                ps2[(j, nn)], lhsT8, w8ks[c][:, :, ts(nn, H)],
                                start=(c == 0), stop=(c == KC - 1),
                                perf_mode=DR,
                            )
                        if c >= 2:
                            cp = c - 2
                            nc.tensor.matmul(
                                ps2[(j, 3)], lhsT8,
                                w8ot[:, 2 * cp : 2 * cp + 2, :],
                                start=False, stop=(cp == 1),
                                perf_mode=DR,
                            )
                for j in (jp, jp + 1):
                    epilogue(
                        [ps2[(j, nn)] for nn in range(4)],
                        cts0[j],
                        slice(j * P, (j + 1) * P),
                        f"g0_{j}",
                    )

            # Slabs 1..7: j-major, full-rate PE against prefetched slabs.
            for g in range(1, BG):
                xs, x8s = xs_n, x8s_n
                if g < BG - 1:
                    xs_n = xpool.tile([P, KO, BG_ROWS], BF16, tag="xs", name=f"xs_{g+1}")
                    nc.sync.dma_start(xs_n[:], xt[g + 1])
                    x8s_n = x8pool.tile([P, KO, BG_ROWS], F8, tag="x8s", name=f"x8s_{g+1}")
                    nc.sync.dma_start(x8s_n[:], x8t[g + 1])
                for j in range(BT_PER_BG):
                    bt = g * BT_PER_BG + j
                    rows = slice(bt * P, (bt + 1) * P)
                    ct = cpool.tile([P, H], BF16, tag="ct", name=f"ct_{bt}")
                    nc.sync.dma_start(ct[:], cell[rows, :])
                    ps = [
                        ppool.tile([P, H], F32, tag="ps", name=f"ps{nn}_{bt}")
                        for nn in range(4)
                    ]
                    last_bt = g == BG - 1 and j == BT_PER_BG - 1
                    mm_tile(
                        ps,
                        lambda k, xs=xs, j=j: xs[:, k, ts(j, P)],
                        lambda c, x8s=x8s, j=j: x8s[:, 2 * c : 2 * c + 2, ts(j, P)],
                        last=last_bt,
                    )
                    epilogue(ps, ct, rows, f"g{g}_{j}",
                             splits=2 if last_bt else 1)

    _split_multi_waits(nc)
    return nc


def kernel(input, cell_state, Wi, bi, Wf, bf, Wg, bg, Wo, bo):
    global LAST_RESULTS

    x = np.asarray(input, dtype=np.float32)
    cell = np.asarray(cell_state, dtype=np.float32)
    Wif = np.concatenate(
        [np.asarray(m, dtype=np.float32) for m in (Wi, Wf)], axis=1
    )  # [D, 2H]
    Wg_ = np.asarray(Wg, dtype=np.float32)
    Wo_ = np.asarray(Wo, dtype=np.float32)
    bcat = np.concatenate(
        [np.asarray(v, dtype=np.float32) for v in (bi, bf, bg, bo)]
    )  # [4H]
    with_bias = bool(np.any(bcat))

    KHD = KH * P  # 512: rows in each K half
    # W -> [p, ko, n], contiguous per partition; k = 128*ko + p.
    w1_dev = np.ascontiguousarray(
        np.concatenate([Wg_[:KHD], Wo_[:KHD] * SO], axis=1)
        .astype(ml_dtypes.bfloat16)
        .reshape(KH, P, NH)
        .transpose(1, 0, 2)
    )
    w2_dev = np.ascontiguousarray(
        Wg_[KHD:].astype(ml_dtypes.bfloat16).reshape(KH, P, H).transpose(1, 0, 2)
    )
    w8_dev = np.ascontiguousarray(
        np.clip(Wif * SW, -240.0, 240.0)
        .astype(ml_dtypes.float8_e4m3)
        .reshape(KO, P, NH)
        .transpose(1, 0, 2)
    )
    w8o_dev = np.ascontiguousarray(
        np.clip(Wo_[KHD:] * SW, -240.0, 240.0)
        .astype(ml_dtypes.float8_e4m3)
        .reshape(KH, P, H)
        .transpose(1, 0, 2)
    )

    in_maps = []
    for c in range(N_CORES):
        xc = x[c * B_LOC : (c + 1) * B_LOC]  # [4096, 1024]
        # -> [bg, p, ko, b] so each 512-row slab DMA is contiguous/partition.
        xt_c = np.ascontiguousarray(
            xc.astype(ml_dtypes.bfloat16)
            .reshape(BG, BG_ROWS, KO, P)
            .transpose(0, 3, 2, 1)
        )
        x8t_c = np.ascontiguousarray(
            np.clip(xc * SX, -240.0, 240.0)
            .astype(ml_dtypes.float8_e4m3)
            .reshape(BG, BG_ROWS, KO, P)
            .transpose(0, 3, 2, 1)
        )
        m = {
            "xt": xt_c,
            "x8t": x8t_c,
            "w1": w1_dev,
            "w2": w2_dev,
            "w8": w8_dev,
            "w8o": w8o_dev,
            "cell": np.ascontiguousarray(
                cell[c * B_LOC : (c + 1) * B_LOC].astype(ml_dtypes.bfloat16)
            ),
        }
        if with_bias:
            m["bias"] = np.ascontiguousarray(
                np.broadcast_to(bcat[None, :], (P, NW)).astype(np.float32)
            )
        in_maps.append(m)

    key = with_bias
    if key not in _CACHED:
        _CACHED[key] = _build(with_bias)
    nc = _CACHED[key]

    trace = os.environ.get("KERNEL_TRACE", "0") == "1"
    res = run_bass_kernel_spmd(nc, in_maps, list(range(N_CORES)), trace=trace)
    LAST_RESULTS = res

    h = np.concatenate([res.results[c]["h_out"] for c in range(N_CORES)], axis=0)
    c_ = np.concatenate([res.results[c]["c_out"] for c in range(N_CORES)], axis=0)
    return h, c_
